# revision 22
# baseline (speedup 1.0000x reference)
"""Trainium2 Bass kernel for PVT-style spatial-reduction attention (v2.1).

Shapes (hardcoded): x [2, 4096, 256], HEAD=8, dh=32, SR=2, R=8, H=W=64.
Sharding: core c = (batch b = c//4, query block j = c%4); per-core x is
pre-rotated so the core's own 1024 query rows are rows 0:1024. The small
conv+LN+KV path is computed redundantly per core (collectives cost more
than they save in this regime).

Structure (cost-model driven):
- LN applied to conv output as explicit center+scale on DVE (2x bf16), so
  the kv projection is a plain 3-matmul accumulation (2 W chunks + rank-8
  shared LoRA); LN rstd and 1/sqrt(dh) are folded upstream so the exp
  input needs no per-tile scale.
- scores in [kv, q] layout; exp split between ScalarE (exact) and DVE
  (int16 Schraudolph exp2 bit-trick).
- PV flipped: pt chunks are the stationary operand, v (+ones column)
  streams at N=33 -> softmax denominators land as psum *columns*;
  reciprocal runs on tiny free dims and the divide is fused into the
  psum drain with a stride-0 free-dim broadcast of the reciprocals.
- attention output transposed back to [ch, q] via PE transposes packed
  into shared psum tiles; projection bias applied via a rank-1 matmul.
- PSUM: "sc" [128,1024] f32 x2 bufs (scores/preamble/transpose/proj) +
  "pv" [128, 3, 132] f32 x4 bufs = exactly 8 banks.
- Emission order keeps PE continuously busy (cost model halves PE clock
  after idle gaps): weights arrive in one packed DMA on the SP queue,
  q-path and strip-1 conv fill PE during the serial LN-stats chains,
  attention on strip-0 kv chunks starts before strip-1 kv is done.
"""
import sys

if "/opt/trn_rl_repo" not in sys.path:
    sys.path.insert(0, "/opt/trn_rl_repo")

import numpy as np
import ml_dtypes

BF16NP = ml_dtypes.bfloat16

HEAD, DH, C, N, B, M, R = 8, 32, 256, 4096, 2, 1024, 8
NB = N // 4          # query rows per core
SCALE = DH ** -0.5
NCORES = 8

# Schraudolph exp2 on bf16 bit layout: bits = round(x*log2e*128 + 16256 - C)
SCH_A = 128.0 * 1.4426950408889634
SCH_B = 16256.0 - 4.0

# packed bf16 weight column offsets
_QW0, _QW1 = 0, 512
_KV0, _KV1 = 512, 1536
_PW0, _PW1 = 1536, 2048
_SR0, _SR1 = 2048, 4096
_AQ0, _AQ1 = 4096, 4112
_AV0, _AV1 = 4112, 4128
_BQ0, _BQ1 = 4128, 4384
_BV0, _BV1 = 4384, 4896
_PB0, _PB1 = 4896, 5152
_WCOLS = 5152

_CACHE = {}


def _act_tile(h01, mc):
    """exp engine assignment: True -> ScalarE exact, False -> DVE Schraudolph."""
    return h01 == 0 or mc % 4 == 0


def _build_program():
    import concourse.bass as bass
    import concourse.tile as tile
    from concourse.bacc import Bacc
    from concourse import mybir, masks

    F32 = mybir.dt.float32
    BF16 = mybir.dt.bfloat16
    I16 = mybir.dt.int16
    I32 = mybir.dt.int32
    AF = mybir.ActivationFunctionType
    ALU = mybir.AluOpType

    nc = Bacc()
    P = 128
    ST = 512       # kv tokens per strip
    MAGIC = 0x5F3759DF

    def bcast_dram(ap, nparts, nfree):
        return bass.AP(tensor=ap.tensor, offset=ap.offset,
                       ap=[[0, nparts], [1, nfree]])

    # ---- DRAM parameters (host-prepped layouts) ----
    xT_d = nc.declare_dram_parameter("xT", [P, 2, N], BF16, isOutput=False)
    wpk_d = nc.declare_dram_parameter("wpk", [P, _WCOLS], BF16, isOutput=False)
    fpk_d = nc.declare_dram_parameter("fpk", [P, 4], F32, isOutput=False)
    out_d = nc.declare_dram_parameter("out", [NB, C], F32, isOutput=True)

    with tile.TileContext(nc) as tc:
        with tc.tile_pool(name="wgt", bufs=1) as WGT, \
             tc.tile_pool(name="acts", bufs=1) as ACTS, \
             tc.tile_pool(name="strips", bufs=2) as STR, \
             tc.tile_pool(name="tmp", bufs=2) as TMP, \
             tc.tile_pool(name="fin", bufs=2) as FIN, \
             tc.tile_pool(name="ps", bufs=1, space="PSUM") as PS, \
             tc.tile_pool(name="dscr", bufs=1, space="DRAM") as DSCR:

            # ---------- input/weight DMAs in priority order ----------
            # (DMA_ENGINES is a serial resource in the cost model: conv
            # weights and the first x strip must land first)
            wpk = WGT.tile([P, _WCOLS], BF16, tag="wpk")
            nc.sync.dma_start(out=wpk[:, _SR0:_SR1], in_=wpk_d[:, _SR0:_SR1])
            xs_ts = []
            for s in range(2):
                xs_t = STR.tile([P, 2, 4 * ST], BF16, tag=f"xs{s}", bufs=1,
                                name=f"xs_t{s}")
                eng = nc.sync if s == 0 else nc.gpsimd
                eng.dma_start(out=xs_t[:],
                              in_=xT_d[:, :, s * 2048:(s + 1) * 2048])
                xs_ts.append(xs_t)
            nc.sync.dma_start(out=wpk[:, 0:_SR0], in_=wpk_d[:, 0:_SR0])
            nc.sync.dma_start(out=wpk[:, _SR1:_WCOLS],
                              in_=wpk_d[:, _SR1:_WCOLS])
            fpk = WGT.tile([P, 4], F32, tag="fpk")
            nc.sync.dma_start(out=fpk[:], in_=fpk_d[:])

            def wslice(c0, c1, shape):
                ap = wpk[:, c0:c1]
                if len(shape) == 2:
                    return ap.rearrange("p (a b) -> p a b", b=shape[1])
                return ap.rearrange("p (a b c) -> p a b c", b=shape[1], c=shape[2])

            qwT = wslice(_QW0, _QW1, (2, C))
            kvwT = wslice(_KV0, _KV1, (2, 2 * C))
            pwT = wslice(_PW0, _PW1, (2, C))
            srwT = wslice(_SR0, _SR1, (2, 4, C))
            aqT = wslice(_AQ0, _AQ1, (2, R))
            avT = wslice(_AV0, _AV1, (2, R))
            bqT = wpk[0:R, _BQ0:_BQ1].rearrange("p (a b) -> p a b", b=P)
            bvT = wpk[0:R, _BV0:_BV1].rearrange("p (a b) -> p a b", b=P)
            pbrow = wpk[0:1, _PB0:_PB1]
            qb = fpk[:, 0:2]
            srb = fpk[:, 2:4]

            ones1 = WGT.tile([P, 1], BF16, tag="ones1")
            nc.gpsimd.memset(ones1[:], 1.0 / C)
            onesq = WGT.tile([1, P], BF16, tag="onesq")
            nc.gpsimd.memset(onesq[:], 1.0)
            ident = WGT.tile([P, P], BF16, tag="ident")
            masks.make_identity(nc, ident[:])

            # persistent activations
            qT = ACTS.tile([P, 2, NB], BF16, tag="qT")
            tq = ACTS.tile([R, NB], BF16, tag="tq")
            kts = ACTS.tile([P, 2, M], BF16, tag="kts")
            vsb = ACTS.tile([P, 8, 2, 4, DH + 1], BF16, tag="vsb")
            nc.gpsimd.memset(vsb[:, :, :, :, DH:DH + 1], 1.0)
            pt = ACTS.tile([P, HEAD, 8, NB], BF16, tag="pt")
            attn_sb = ACTS.tile([P, 8, 2, P], BF16, tag="attn_sb")
            outT = ACTS.tile([P, 2, NB], BF16, tag="outT")
            rec = ACTS.tile([P, 2, 3, 3, 4], F32, tag="rec")

            xsbs, xsss, stats_st, kv_t2 = {}, {}, {}, {}

            # =============== preamble stages ===============
            def conv_stage(s):
                xs_t = xs_ts[s]
                cvt = PS.tile([P, 1024], F32, tag="sc", bufs=2, name="cvt")
                for oc in range(2):
                    cps = cvt[:, oc * ST:(oc + 1) * ST]
                    first = True
                    for cc in range(2):
                        xv = xs_t[:, cc, :].rearrange(
                            "p (i a j b) -> p i a j b", i=16, a=2, j=32, b=2)
                        for di in range(2):
                            for dj in range(2):
                                nc.tensor.matmul(
                                    cps, srwT[:, cc, di * 2 + dj,
                                              oc * P:(oc + 1) * P],
                                    xv[:, :, di, :, dj],
                                    start=first,
                                    stop=(cc == 1 and di == 1 and dj == 1))
                                first = False
                xsb = STR.tile([P, 2, ST], BF16, tag="xsb", name="xsb")
                nc.scalar.activation(out=xsb[:, 0, :], in_=cvt[:, 0:ST],
                                     func=AF.Identity, bias=srb[:, 0:1])
                nc.scalar.activation(out=xsb[:, 1, :], in_=cvt[:, ST:2 * ST],
                                     func=AF.Identity, bias=srb[:, 1:2])
                xsbs[s] = xsb

            def stats_a(s):
                xsb = xsbs[s]
                sq = STR.tile([P, 2, ST], BF16, tag="sq", name="sq", bufs=1)
                nc.vector.tensor_tensor(out=sq[:], in0=xsb[:], in1=xsb[:],
                                        op=ALU.mult)
                stt = PS.tile([P, 1024], F32, tag="sc", bufs=2, name="stt")
                sxp = stt[0:1, 0:ST]
                nc.tensor.matmul(sxp, ones1[:], xsb[:, 0, :], start=True, stop=False)
                nc.tensor.matmul(sxp, ones1[:], xsb[:, 1, :], start=False, stop=True)
                sxxp = stt[0:1, ST:2 * ST]
                nc.tensor.matmul(sxxp, ones1[:], sq[:, 0, :], start=True, stop=False)
                nc.tensor.matmul(sxxp, ones1[:], sq[:, 1, :], start=False, stop=True)
                negmu = TMP.tile([1, ST], BF16, tag="negmu", name="negmu")
                nc.vector.tensor_scalar_mul(out=negmu[:], in0=sxp, scalar1=-1.0)
                ex2_sb = TMP.tile([1, ST], F32, tag="ex2sb", name="ex2_sb")
                nc.vector.tensor_copy(out=ex2_sb[:], in_=sxxp)

                # chunk-major repack [1,512] -> [128,4] via DRAM bounce
                nm_d = DSCR.tile([ST], BF16, tag=f"nm{s}", name="nm_d")
                nc.sync.dma_start(out=nm_d[:], in_=negmu[:])
                ex_d = DSCR.tile([ST], F32, tag=f"ex{s}", name="ex_d")
                nc.sync.dma_start(out=ex_d[:], in_=ex2_sb[:])
                mur = TMP.tile([P, 4], BF16, tag="mur", name="mur")
                nc.sync.dma_start(out=mur[:],
                                  in_=nm_d[:].rearrange("(g p) -> p g", p=P))
                ex2r = TMP.tile([P, 4], F32, tag="ex2r", name="ex2r")
                nc.sync.dma_start(out=ex2r[:],
                                  in_=ex_d[:].rearrange("(g p) -> p g", p=P))
                stats_st[s] = (mur, ex2r, nm_d)

            def stats_b(s):
                mur, ex2r, nm_d = stats_st[s]
                xsb = xsbs[s]
                # rstd via quake rsqrt (1 newton step); an = rstd (bf16)
                nmu2 = TMP.tile([P, 4], F32, tag="nmu2", name="nmu2")
                nc.vector.scalar_tensor_tensor(out=nmu2[:], in0=mur[:], scalar=-1.0,
                                               in1=mur[:], op0=ALU.mult, op1=ALU.mult)
                ve = TMP.tile([P, 4], F32, tag="ve", name="ve")
                nc.vector.scalar_tensor_tensor(out=ve[:], in0=nmu2[:], scalar=1e-5,
                                               in1=ex2r[:], op0=ALU.add, op1=ALU.add)
                hsh = TMP.tile([P, 4], I32, tag="hsh", name="hsh")
                nc.vector.tensor_scalar(out=hsh[:], in0=ve[:].bitcast(I32), scalar1=1,
                                        scalar2=None, op0=ALU.logical_shift_right)
                nc.vector.tensor_scalar(out=hsh[:], in0=hsh[:], scalar1=-1,
                                        scalar2=MAGIC, op0=ALU.mult, op1=ALU.add)
                y0 = hsh[:].bitcast(F32)
                nt = TMP.tile([P, 4], F32, tag="nt", name="nt")
                nc.vector.tensor_mul(out=nt[:], in0=y0, in1=y0)
                nc.vector.scalar_tensor_tensor(out=nt[:], in0=nt[:], scalar=-0.5,
                                               in1=ve[:], op0=ALU.mult, op1=ALU.mult)
                nc.vector.tensor_scalar_add(out=nt[:], in0=nt[:], scalar1=1.5)
                an_s = TMP.tile([P, 4], BF16, tag="an_s", name="an_s")
                nc.vector.tensor_mul(out=an_s[:], in0=y0, in1=nt[:])
                # token-major rstd broadcast tile [128, ST]
                an_d = DSCR.tile([ST], BF16, tag=f"an{s}", name="an_d")
                nc.sync.dma_start(out=an_d[:].rearrange("(g p) -> p g", p=P),
                                  in_=an_s[:])
                an_free = STR.tile([P, ST], BF16, tag="an_free", name="an_free", bufs=1)
                nc.sync.dma_start(out=an_free[:], in_=bcast_dram(an_d[:], P, ST))
                mu_free = STR.tile([P, ST], BF16, tag="mu_free", name="mu_free")
                nc.sync.dma_start(out=mu_free[:], in_=bcast_dram(nm_d[:], P, ST))

                # LN-normalized activations (centered + scaled), bf16 2x ops
                xsc = STR.tile([P, 2, ST], BF16, tag="xsc", name="xsc", bufs=1)
                mub = bass.AP(tensor=mu_free.tensor, offset=mu_free[:].offset,
                              ap=[list(mu_free[:].ap[0]), [0, 2],
                                  list(mu_free[:].ap[1])])
                nc.vector.tensor_tensor(out=xsc[:], in0=xsb[:], in1=mub, op=ALU.add)
                xss = STR.tile([P, 2, ST], BF16, tag="xss", name="xss")
                anb = bass.AP(tensor=an_free.tensor, offset=an_free[:].offset,
                              ap=[list(an_free[:].ap[0]), [0, 2],
                                  list(an_free[:].ap[1])])
                nc.vector.tensor_tensor(out=xss[:], in0=xsc[:], in1=anb, op=ALU.mult)
                xsss[s] = xss

            def kv_stage(s, part=None):
                xss = xsss[s]
                if part == 1:
                    kv_vpart(s)
                    return
                # t2 (shared kv lora, rank 8) from normalized input
                t2sb = STR.tile([R, ST], BF16, tag="t2sb", name="t2sb")
                kvp = PS.tile([P, 1024], F32, tag="sc", bufs=2, name="kvp")
                t2p = kvp[0:R, 0:ST]
                nc.tensor.matmul(t2p, avT[:, 0, :], xss[:, 0, :], start=True, stop=False)
                nc.tensor.matmul(t2p, avT[:, 1, :], xss[:, 1, :], start=False, stop=True)
                nc.scalar.activation(out=t2sb[:], in_=t2p, func=AF.Copy)

                # k projection: 2 out-chunks x (2 W chunks + rank-8 lora)
                kvq0 = PS.tile([P, 1024], F32, tag="sc", bufs=2, name="kvq0")
                kv_ps = [kvp[:, ST:2 * ST], kvq0[:, 0:ST]]
                for kvoc in range(2):
                    kps = kv_ps[kvoc]
                    nc.tensor.matmul(kps, kvwT[:, 0, kvoc * P:(kvoc + 1) * P],
                                     xss[:, 0, :], start=True, stop=False)
                    nc.tensor.matmul(kps, kvwT[:, 1, kvoc * P:(kvoc + 1) * P],
                                     xss[:, 1, :], start=False, stop=False)
                    nc.tensor.matmul(kps, bvT[:, kvoc, :], t2sb[:],
                                     start=False, stop=True)
                    if kvoc == 0:
                        nc.scalar.activation(out=kts[:, 0, s * ST:(s + 1) * ST],
                                             in_=kps, func=AF.Copy)
                    else:
                        nc.vector.tensor_copy(out=kts[:, 1, s * ST:(s + 1) * ST],
                                              in_=kps)
                kv_t2[s] = t2sb
                if part is None:
                    kv_vpart(s)

            def kv_vpart(s):
                xss = xsss[s]
                t2sb = kv_t2[s]
                vtmp = STR.tile([P, 2, ST], BF16, tag="vtmp", name="vtmp")
                kvq1 = PS.tile([P, 1024], F32, tag="sc", bufs=2, name="kvq1")
                kv_ps = [kvq1[:, 0:ST], kvq1[:, ST:2 * ST]]
                for vc in range(2):
                    kvoc = vc + 2
                    kps = kv_ps[vc]
                    nc.tensor.matmul(kps, kvwT[:, 0, kvoc * P:(kvoc + 1) * P],
                                     xss[:, 0, :], start=True, stop=False)
                    nc.tensor.matmul(kps, kvwT[:, 1, kvoc * P:(kvoc + 1) * P],
                                     xss[:, 1, :], start=False, stop=False)
                    nc.tensor.matmul(kps, bvT[:, kvoc, :], t2sb[:],
                                     start=False, stop=True)
                    if vc == 0:
                        nc.scalar.activation(out=vtmp[:, 0, :], in_=kps,
                                             func=AF.Copy)
                    else:
                        nc.vector.tensor_copy(out=vtmp[:, 1, :], in_=kps)

                # v transpose: 8 PE transposes packed into one psum tile,
                # then one 2x bf16 drain into vsb (ones column pre-set)
                vtp = PS.tile([P, 1024], F32, tag="sc", bufs=2, name="vtp")
                vtb = vtp[:].bitcast(BF16).rearrange("p (u v) -> p u v", v=P)
                for vc in range(2):
                    for u4 in range(4):
                        nc.tensor.transpose(vtb[:, vc * 4 + u4, :],
                                            vtmp[:, vc, u4 * P:(u4 + 1) * P],
                                            ident[:])
                # vsb dims [P, mc 8, vc 2, h' 4, d 33]; iterate (vc, u4, h, d)
                dst = bass.AP(
                    tensor=vsb.tensor, offset=vsb[:, 4 * s, 0, 0, 0].offset,
                    ap=[list(vsb[:].ap[0]),
                        [(DH + 1) * 4, 2], [(DH + 1) * 4 * 2, 4],
                        [DH + 1, 4], [1, DH]])
                nc.vector.tensor_copy(
                    out=dst,
                    in_=vtb[:, 0:8, :].rearrange("p s (h d) -> p s h d", d=DH))

            # =============== q path (needs strip0 only) ===============
            def q_path():
                xs_t = xs_ts[0]
                tqp_t = PS.tile([P, 1024], F32, tag="sc", bufs=2, name="tqp_t")
                for nh in range(2):
                    sl = slice(nh * 512, (nh + 1) * 512)
                    tqp = tqp_t[0:R, sl]
                    nc.tensor.matmul(tqp, aqT[:, 0, :], xs_t[:, 0, sl],
                                     start=True, stop=False)
                    nc.tensor.matmul(tqp, aqT[:, 1, :], xs_t[:, 1, sl],
                                     start=False, stop=True)
                nc.scalar.activation(out=tq[:], in_=tqp_t[0:R, :], func=AF.Copy)
                for oc in range(2):
                    qps = PS.tile([P, 1024], F32, tag="sc", bufs=2, name="qps")
                    for nh in range(2):
                        sl = slice(nh * 512, (nh + 1) * 512)
                        nc.tensor.matmul(qps[:, sl],
                                         qwT[:, 0, oc * P:(oc + 1) * P],
                                         xs_t[:, 0, sl], start=True, stop=False)
                        nc.tensor.matmul(qps[:, sl],
                                         qwT[:, 1, oc * P:(oc + 1) * P],
                                         xs_t[:, 1, sl], start=False, stop=False)
                        nc.tensor.matmul(qps[:, sl], bqT[:, oc, :], tq[:, sl],
                                         start=False, stop=True)
                    nc.scalar.activation(out=qT[:, oc, :], in_=qps[:],
                                         func=AF.Identity, bias=qb[:, oc:oc + 1])

            # =============== attention ===============
            pv_state = {}
            pv_tiles = {}

            def attn_scores_exp(ghalf, g2, mc):
                for h01 in range(2):
                    hh = 2 * g2 + h01          # head within half (0..3)
                    h = 4 * ghalf + hh
                    sc = PS.tile([P, 1024], F32, tag="sc", bufs=2, name="sc")
                    lhsT = kts[32 * hh:32 * hh + 32, ghalf, mc * P:(mc + 1) * P]
                    for nh in range(2):
                        sl = slice(nh * 512, (nh + 1) * 512)
                        nc.tensor.matmul(sc[:, sl], lhsT,
                                         qT[32 * hh:32 * hh + 32, ghalf, sl],
                                         start=True, stop=True,
                                         tile_position=(32 * hh, 0))
                    if _act_tile(h01, mc):
                        nc.scalar.activation(out=pt[:, h, mc, :], in_=sc[:],
                                             func=AF.Exp)
                    else:
                        nc.vector.tensor_scalar(
                            out=pt[:, h, mc, :].bitcast(I16), in0=sc[:],
                            scalar1=SCH_A, scalar2=SCH_B,
                            op0=ALU.mult, op1=ALU.add)

            def attn_pv(ghalf, g2, mc, qcs=range(8)):
                for h01 in range(2):
                    hh = 2 * g2 + h01
                    h = 4 * ghalf + hh
                    for qc in qcs:
                        bank = pv_tiles[(ghalf, qc // 3)]
                        st = pv_state[(ghalf, qc // 3)]
                        nc.tensor.matmul(
                            bank[:, qc % 3, 33 * hh:33 * hh + 33],
                            pt[:, h, mc, qc * P:(qc + 1) * P],
                            vsb[:, mc, ghalf, hh, :],
                            start=(st["n"] == 0),
                            stop=(st["n"] == st["total"] - 1),
                            skip_group_check=True)
                        st["n"] += 1

            def attn_tail(ghalf):
                # reciprocals of the denominator columns, then fused
                # divide + psum drain into attn_sb
                for b3 in range(3):
                    nq = 3 if b3 < 2 else 2
                    bank = pv_tiles[(ghalf, b3)]
                    bap = bank[:].rearrange("p q (h d) -> p q h d", d=DH + 1)
                    r = rec[:, ghalf, b3, 0:nq, :]
                    nc.vector.reciprocal(out=r, in_=bap[:, 0:nq, :, DH])
                    rb = bass.AP(tensor=rec.tensor, offset=r.offset,
                                 ap=[list(d) for d in r.ap] + [[0, DH]])
                    dst = bass.AP(
                        tensor=attn_sb.tensor,
                        offset=attn_sb[:, 3 * b3, ghalf, 0].offset,
                        ap=[list(attn_sb[:].ap[0]), [2 * P, nq], [DH, 4], [1, DH]])
                    nc.vector.tensor_tensor(out=dst, in0=bap[:, 0:nq, :, 0:DH],
                                            in1=rb, op=ALU.mult)

            def attn_transpose(ghalf, qp):
                # attn_sb [q, c-block] -> outT [c, q]; 2 PE transposes packed
                # per sc tile (bf16), one 2x drain (split ACT/DVE)
                tp = PS.tile([P, 1024], F32, tag="sc", bufs=2, name="tp")
                tb = tp[:].bitcast(BF16).rearrange("p (u v) -> p u v", v=P)
                for i in range(2):
                    nc.tensor.transpose(tb[:, i, :],
                                        attn_sb[:, 2 * qp + i, ghalf, :],
                                        ident[:])
                dst = bass.AP(
                    tensor=outT.tensor,
                    offset=outT[:, ghalf, 2 * qp * P].offset,
                    ap=[list(outT[:].ap[0]), [P, 2], [1, P]])
                if qp % 2 == 0:
                    nc.scalar.activation(out=dst, in_=tb[:, 0:2, :], func=AF.Copy)
                else:
                    nc.vector.tensor_copy(out=dst, in_=tb[:, 0:2, :])

            def proj_out(t8):
                pp_t = PS.tile([P, 3, 132], F32, tag="pv", bufs=4, name="pp_t")
                pp = pp_t[:].bitcast(F32).rearrange("p a b -> p (a b)")[:, 0:C]
                nc.tensor.matmul(pp, outT[:, 0, t8 * P:(t8 + 1) * P],
                                 pwT[:, 0, :], start=True, stop=False)
                nc.tensor.matmul(pp, outT[:, 1, t8 * P:(t8 + 1) * P],
                                 pwT[:, 1, :], start=False, stop=False)
                nc.tensor.matmul(pp, onesq[:], pbrow, start=False, stop=True)
                fin = FIN.tile([P, C], F32, tag="fin", name="fin", bufs=3)
                if t8 % 2 == 0:
                    nc.scalar.activation(out=fin[:], in_=pp, func=AF.Copy)
                else:
                    nc.vector.tensor_copy(out=fin[:], in_=pp)
                nc.sync.dma_start(out=out_d[t8 * P:(t8 + 1) * P, :], in_=fin[:])

            # =============== emission schedule ===============
            def pv_alloc(ghalf):
                for b3 in range(3):
                    t = PS.tile([P, 3, 132], F32, tag="pv", bufs=4,
                                name=f"pv{ghalf}{b3}")
                    pv_tiles[(ghalf, b3)] = t
                    nq = 3 if b3 < 2 else 2
                    pv_state[(ghalf, b3)] = {"n": 0, "total": nq * 4 * 8}

            conv_stage(0)
            stats_a(0)
            q_path()                 # PE-heavy; runs while LN chains complete
            stats_b(0)
            kv_stage(0)
            conv_stage(1)
            stats_a(1)
            pv_alloc(0)
            pv_alloc(1)

            # software-pipelined attention: pv matmuls lag scores/exp by one
            # step so the in-order PE queue never parks on an exp result.
            # Both ghalves' strip-0 steps run before the strip-1 kv
            # projection (whose LN DMA chain is slow); gh1's pv work is
            # restricted to its conflict-free psum bank until gh0's banks
            # drain at tail(0).
            steps = ([(0, g2, mc) for g2 in range(2) for mc in range(4)]
                     + [(1, g2, mc) for g2 in range(2) for mc in range(4)]
                     + ["kv1a",
                        (0, 0, 4)]
                     + ["kv1b"]
                     + [(0, g2, mc) for g2 in range(2) for mc in range(4, 8)][1:]
                     + ["tail0"]
                     + [(1, g2, mc) for g2 in range(2) for mc in range(4, 8)])
            pvq = []            # lagged pv batches
            gh1_stash = []      # gh1 batches awaiting banks 1,2
            transq = []         # deferred gh0 transposes
            gh0_open = True
            cnt = 0

            def pump(budget=1):
                n = 0
                while pvq and n < budget:
                    it = pvq[0]
                    if it[0] == 0:
                        attn_pv(*it)
                        pvq.pop(0)
                    elif gh0_open:
                        attn_pv(*it, qcs=range(3))
                        gh1_stash.append(it)
                        pvq.pop(0)
                    else:
                        attn_pv(*it)
                        pvq.pop(0)
                    n += 1

            for item in steps:
                if item == "kv1a":
                    kv_stage(1, part=0)
                    continue
                if item == "kv1b":
                    kv_stage(1, part=1)
                    continue
                if item == "tail0":
                    pump(budget=8)      # flush all remaining gh0 batches
                    attn_tail(0)
                    gh0_open = False
                    transq = [(0, qp) for qp in range(4)]
                    continue
                attn_scores_exp(*item)
                pump()
                if not gh0_open and gh1_stash:
                    it = gh1_stash.pop(0)
                    attn_pv(*it, qcs=range(3, 8))
                cnt += 1
                if cnt == 6:
                    stats_b(1)
                if transq and cnt % 2 == 0:
                    attn_transpose(*transq.pop(0))
                pvq.append(item)
            pump(budget=8)
            while gh1_stash:
                attn_pv(*gh1_stash.pop(0), qcs=range(3, 8))
            while transq:
                attn_transpose(*transq.pop(0))
            attn_tail(1)
            for qp in range(4):
                attn_transpose(1, qp)
                proj_out(2 * qp)
                proj_out(2 * qp + 1)

    nc.finalize()
    return nc


def _prep_shared(q_w, q_b, kv_w, kv_b, proj_w, proj_b, a_q, b_q, a_v, b_v,
                 sr_w, sr_b, ln_g, ln_b):
    f32 = np.float32

    def chunkT(w):  # [in, out] -> [128, n_in_chunks * out] (chunk-major cols)
        wt = np.ascontiguousarray(np.asarray(w, f32).T)
        ic, oc = wt.shape
        return np.ascontiguousarray(
            wt.reshape(ic // 128, 128, oc).transpose(1, 0, 2)).reshape(128, -1)

    kv_w = np.asarray(kv_w, f32)
    a_v = np.asarray(a_v, f32)
    b_v = np.asarray(b_v, f32)
    g = np.asarray(ln_g, f32)
    bb = np.asarray(ln_b, f32)
    proj_w = np.asarray(proj_w, f32)
    # fold LayerNorm gamma into kv/a_v weights; LN mean/rstd applied on-chip;
    # k-side beta constants dropped (softmax shift invariance), v-side folded
    # into the projection bias. 1/sqrt(dh) folded into q weights.
    Wg = kv_w * g[None, :]
    Avg = a_v * g[None, :]
    wbt = kv_w @ bb + np.asarray(kv_b, f32)
    dconst = b_v @ (a_v @ bb)
    wv_const = wbt[C:] + dconst
    pb_eff = np.asarray(proj_b, f32) + proj_w @ wv_const

    srwT = np.asarray(sr_w, f32).transpose(1, 2, 3, 0).reshape(2, 128, 4, C)
    srwT = np.ascontiguousarray(srwT.transpose(1, 0, 2, 3)).reshape(128, -1)
    bqT = (np.asarray(b_q, f32) * SCALE).T.reshape(R, 2 * 128)
    bvT2 = b_v.T.reshape(R, 2, 128)      # [R, kv-half-chunk, 128]
    bvT = np.zeros((R, 4 * 128), f32)
    for kvoc in range(4):
        bvT[:, kvoc * 128:(kvoc + 1) * 128] = bvT2[:, kvoc % 2, :]

    wpk = np.zeros((128, _WCOLS), f32)
    wpk[:, _QW0:_QW1] = chunkT(np.asarray(q_w, f32) * SCALE)
    wpk[:, _KV0:_KV1] = chunkT(Wg)
    wpk[:, _PW0:_PW1] = chunkT(proj_w)
    wpk[:, _SR0:_SR1] = srwT
    wpk[:, _AQ0:_AQ1] = chunkT(a_q)
    wpk[:, _AV0:_AV1] = chunkT(Avg)
    wpk[0:R, _BQ0:_BQ1] = bqT
    wpk[0:R, _BV0:_BV1] = bvT
    wpk[0:1, _PB0:_PB1] = pb_eff.reshape(1, C)

    fpk = np.zeros((128, 4), f32)
    fpk[:, 0:2] = (np.asarray(q_b, f32) * SCALE).reshape(2, 128).T
    fpk[:, 2:4] = np.asarray(sr_b, f32).reshape(2, 128).T
    return dict(wpk=np.ascontiguousarray(wpk).astype(BF16NP),
                fpk=np.ascontiguousarray(fpk))


def kernel(x, q_w, q_b, kv_w, kv_b, proj_w, proj_b, a_q, b_q, a_v, b_v,
           sr_w, sr_b, ln_g, ln_b, H, W):
    from concourse.bass_utils import run_bass_kernel_spmd

    x = np.asarray(x, np.float32)
    assert x.shape == (B, N, C) and int(H) == 64 and int(W) == 64

    if "nc" not in _CACHE:
        _CACHE["nc"] = _build_program()
    nc = _CACHE["nc"]

    shared = _prep_shared(q_w, q_b, kv_w, kv_b, proj_w, proj_b, a_q, b_q,
                          a_v, b_v, sr_w, sr_b, ln_g, ln_b)
    in_maps = []
    for c in range(NCORES):
        b, j = c // 4, c % 4
        xb = np.roll(x[b], -NB * j, axis=0)          # own block at rows 0:1024
        xT = np.ascontiguousarray(xb.T.astype(BF16NP))  # [256, 4096]
        xT = np.ascontiguousarray(
            xT.reshape(2, 128, N).transpose(1, 0, 2))   # [128, 2, 4096]
        in_maps.append(dict(shared, xT=xT))

    res = run_bass_kernel_spmd(nc, in_maps, list(range(NCORES)))
    out = np.empty((B, N, C), np.float32)
    for c in range(NCORES):
        b, j = c // 4, c % 4
        out[b, NB * j:NB * (j + 1)] = res.results[c]["out"]
    return out


# revision 23
# speedup vs baseline: 1.0344x; 1.0344x over previous
"""Trainium2 Bass kernel for PVT-style spatial-reduction attention (v2.1).

Shapes (hardcoded): x [2, 4096, 256], HEAD=8, dh=32, SR=2, R=8, H=W=64.
Sharding: core c = (batch b = c//4, query block j = c%4); per-core x is
pre-rotated so the core's own 1024 query rows are rows 0:1024. The small
conv+LN+KV path is computed redundantly per core (collectives cost more
than they save in this regime).

Structure (cost-model driven):
- LN applied to conv output as explicit center+scale on DVE (2x bf16), so
  the kv projection is a plain 3-matmul accumulation (2 W chunks + rank-8
  shared LoRA); LN rstd and 1/sqrt(dh) are folded upstream so the exp
  input needs no per-tile scale.
- scores in [kv, q] layout; exp split between ScalarE (exact) and DVE
  (int16 Schraudolph exp2 bit-trick).
- PV flipped: pt chunks are the stationary operand, v (+ones column)
  streams at N=33 -> softmax denominators land as psum *columns*;
  reciprocal runs on tiny free dims and the divide is fused into the
  psum drain with a stride-0 free-dim broadcast of the reciprocals.
- attention output transposed back to [ch, q] via PE transposes packed
  into shared psum tiles; projection bias applied via a rank-1 matmul.
- PSUM: "sc" [128,1024] f32 x2 bufs (scores/preamble/transpose/proj) +
  "pv" [128, 3, 132] f32 x4 bufs = exactly 8 banks.
- Emission order keeps PE continuously busy (cost model halves PE clock
  after idle gaps): weights arrive in one packed DMA on the SP queue,
  q-path and strip-1 conv fill PE during the serial LN-stats chains,
  attention on strip-0 kv chunks starts before strip-1 kv is done.
"""
import sys

if "/opt/trn_rl_repo" not in sys.path:
    sys.path.insert(0, "/opt/trn_rl_repo")

import numpy as np
import ml_dtypes

BF16NP = ml_dtypes.bfloat16

HEAD, DH, C, N, B, M, R = 8, 32, 256, 4096, 2, 1024, 8
NB = N // 4          # query rows per core
SCALE = DH ** -0.5
NCORES = 8

# Schraudolph exp2 on bf16 bit layout: bits = round(x*log2e*128 + 16256 - C)
SCH_A = 128.0 * 1.4426950408889634
SCH_B = 16256.0 - 4.0

# packed bf16 weight column offsets
_QW0, _QW1 = 0, 512
_KV0, _KV1 = 512, 1536
_PW0, _PW1 = 1536, 2048
_SR0, _SR1 = 2048, 4096
_AQ0, _AQ1 = 4096, 4112
_AV0, _AV1 = 4112, 4128
_BQ0, _BQ1 = 4128, 4384
_BV0, _BV1 = 4384, 4896
_PB0, _PB1 = 4896, 5152
_WCOLS = 5152

_CACHE = {}


def _act_tile(h01, mc):
    """exp engine assignment: True -> ScalarE exact, False -> DVE Schraudolph."""
    return h01 == 0 or mc % 4 == 0


def _build_program():
    import concourse.bass as bass
    import concourse.tile as tile
    from concourse.bacc import Bacc
    from concourse import mybir, masks

    F32 = mybir.dt.float32
    BF16 = mybir.dt.bfloat16
    I16 = mybir.dt.int16
    I32 = mybir.dt.int32
    AF = mybir.ActivationFunctionType
    ALU = mybir.AluOpType

    nc = Bacc()
    P = 128
    ST = 512       # kv tokens per strip
    MAGIC = 0x5F3759DF

    def bcast_dram(ap, nparts, nfree):
        return bass.AP(tensor=ap.tensor, offset=ap.offset,
                       ap=[[0, nparts], [1, nfree]])

    # ---- DRAM parameters (host-prepped layouts) ----
    xT_d = nc.declare_dram_parameter("xT", [P, 2, N], BF16, isOutput=False)
    wpk_d = nc.declare_dram_parameter("wpk", [P, _WCOLS], BF16, isOutput=False)
    fpk_d = nc.declare_dram_parameter("fpk", [P, 4], F32, isOutput=False)
    out_d = nc.declare_dram_parameter("out", [NB, C], F32, isOutput=True)

    with tile.TileContext(nc) as tc:
        with tc.tile_pool(name="wgt", bufs=1) as WGT, \
             tc.tile_pool(name="acts", bufs=1) as ACTS, \
             tc.tile_pool(name="strips", bufs=2) as STR, \
             tc.tile_pool(name="tmp", bufs=2) as TMP, \
             tc.tile_pool(name="fin", bufs=2) as FIN, \
             tc.tile_pool(name="ps", bufs=1, space="PSUM") as PS, \
             tc.tile_pool(name="dscr", bufs=1, space="DRAM") as DSCR:

            # ---------- input/weight DMAs in priority order ----------
            # (DMA_ENGINES is a serial resource in the cost model: conv
            # weights and the first x strip must land first)
            wpk = WGT.tile([P, _WCOLS], BF16, tag="wpk")
            nc.sync.dma_start(out=wpk[:, _SR0:_SR1], in_=wpk_d[:, _SR0:_SR1])
            xs_ts = []
            for s in range(2):
                xs_t = STR.tile([P, 2, 4 * ST], BF16, tag=f"xs{s}", bufs=1,
                                name=f"xs_t{s}")
                eng = nc.sync if s == 0 else nc.gpsimd
                eng.dma_start(out=xs_t[:],
                              in_=xT_d[:, :, s * 2048:(s + 1) * 2048])
                xs_ts.append(xs_t)
            nc.sync.dma_start(out=wpk[:, 0:_SR0], in_=wpk_d[:, 0:_SR0])
            nc.sync.dma_start(out=wpk[:, _SR1:_WCOLS],
                              in_=wpk_d[:, _SR1:_WCOLS])
            fpk = WGT.tile([P, 4], F32, tag="fpk")
            nc.sync.dma_start(out=fpk[:], in_=fpk_d[:])

            def wslice(c0, c1, shape):
                ap = wpk[:, c0:c1]
                if len(shape) == 2:
                    return ap.rearrange("p (a b) -> p a b", b=shape[1])
                return ap.rearrange("p (a b c) -> p a b c", b=shape[1], c=shape[2])

            qwT = wslice(_QW0, _QW1, (2, C))
            kvwT = wslice(_KV0, _KV1, (2, 2 * C))
            pwT = wslice(_PW0, _PW1, (2, C))
            srwT = wslice(_SR0, _SR1, (2, 4, C))
            aqT = wslice(_AQ0, _AQ1, (2, R))
            avT = wslice(_AV0, _AV1, (2, R))
            bqT = wpk[0:R, _BQ0:_BQ1].rearrange("p (a b) -> p a b", b=P)
            bvT = wpk[0:R, _BV0:_BV1].rearrange("p (a b) -> p a b", b=P)
            pbrow = wpk[0:1, _PB0:_PB1]
            qb = fpk[:, 0:2]
            srb = fpk[:, 2:4]

            ones1 = WGT.tile([P, 1], BF16, tag="ones1")
            nc.gpsimd.memset(ones1[:], 1.0 / C)
            onesq = WGT.tile([1, P], BF16, tag="onesq")
            nc.gpsimd.memset(onesq[:], 1.0)
            ident = WGT.tile([P, P], BF16, tag="ident")
            masks.make_identity(nc, ident[:])

            # persistent activations
            qT = ACTS.tile([P, 2, NB], BF16, tag="qT")
            tq = ACTS.tile([R, NB], BF16, tag="tq")
            kts = ACTS.tile([P, 2, M], BF16, tag="kts")
            vsb = ACTS.tile([P, 8, 2, 4, DH + 1], BF16, tag="vsb")
            nc.gpsimd.memset(vsb[:, :, :, :, DH:DH + 1], 1.0)
            pt = ACTS.tile([P, HEAD, 8, NB], BF16, tag="pt")
            attn_sb = ACTS.tile([P, 8, 2, P], BF16, tag="attn_sb")
            outT = ACTS.tile([P, 2, NB], BF16, tag="outT")
            rec = ACTS.tile([P, 2, 3, 3, 4], F32, tag="rec")

            xsbs, xsss, stats_st, kv_t2 = {}, {}, {}, {}

            # =============== preamble stages ===============
            def conv_stage(s):
                xs_t = xs_ts[s]
                cvt = PS.tile([P, 1024], F32, tag="sc", bufs=2, name="cvt")
                for oc in range(2):
                    cps = cvt[:, oc * ST:(oc + 1) * ST]
                    first = True
                    for cc in range(2):
                        xv = xs_t[:, cc, :].rearrange(
                            "p (i a j b) -> p i a j b", i=16, a=2, j=32, b=2)
                        for di in range(2):
                            for dj in range(2):
                                nc.tensor.matmul(
                                    cps, srwT[:, cc, di * 2 + dj,
                                              oc * P:(oc + 1) * P],
                                    xv[:, :, di, :, dj],
                                    start=first,
                                    stop=(cc == 1 and di == 1 and dj == 1))
                                first = False
                xsb = STR.tile([P, 2, ST], BF16, tag="xsb", name="xsb")
                nc.scalar.activation(out=xsb[:, 0, :], in_=cvt[:, 0:ST],
                                     func=AF.Identity, bias=srb[:, 0:1])
                nc.scalar.activation(out=xsb[:, 1, :], in_=cvt[:, ST:2 * ST],
                                     func=AF.Identity, bias=srb[:, 1:2])
                xsbs[s] = xsb

            def stats_a(s):
                xsb = xsbs[s]
                sq = STR.tile([P, 2, ST], BF16, tag="sq", name="sq", bufs=1)
                nc.vector.tensor_tensor(out=sq[:], in0=xsb[:], in1=xsb[:],
                                        op=ALU.mult)
                stt = PS.tile([P, 1024], F32, tag="sc", bufs=2, name="stt")
                sxp = stt[0:1, 0:ST]
                nc.tensor.matmul(sxp, ones1[:], xsb[:, 0, :], start=True, stop=False)
                nc.tensor.matmul(sxp, ones1[:], xsb[:, 1, :], start=False, stop=True)
                sxxp = stt[0:1, ST:2 * ST]
                nc.tensor.matmul(sxxp, ones1[:], sq[:, 0, :], start=True, stop=False)
                nc.tensor.matmul(sxxp, ones1[:], sq[:, 1, :], start=False, stop=True)
                negmu = TMP.tile([1, ST], BF16, tag="negmu", name="negmu")
                nc.vector.tensor_scalar_mul(out=negmu[:], in0=sxp, scalar1=-1.0)
                ex2_sb = TMP.tile([1, ST], F32, tag="ex2sb", name="ex2_sb")
                nc.vector.tensor_copy(out=ex2_sb[:], in_=sxxp)

                # chunk-major repack [1,512] -> [128,4] via DRAM bounce
                nm_d = DSCR.tile([ST], BF16, tag=f"nm{s}", name="nm_d")
                nc.sync.dma_start(out=nm_d[:], in_=negmu[:])
                ex_d = DSCR.tile([ST], F32, tag=f"ex{s}", name="ex_d")
                nc.sync.dma_start(out=ex_d[:], in_=ex2_sb[:])
                mur = TMP.tile([P, 4], BF16, tag="mur", name="mur")
                nc.sync.dma_start(out=mur[:],
                                  in_=nm_d[:].rearrange("(g p) -> p g", p=P))
                ex2r = TMP.tile([P, 4], F32, tag="ex2r", name="ex2r")
                nc.sync.dma_start(out=ex2r[:],
                                  in_=ex_d[:].rearrange("(g p) -> p g", p=P))
                stats_st[s] = (mur, ex2r, nm_d)

            def stats_b(s):
                mur, ex2r, nm_d = stats_st[s]
                xsb = xsbs[s]
                # rstd via quake rsqrt (1 newton step); an = rstd (bf16)
                nmu2 = TMP.tile([P, 4], F32, tag="nmu2", name="nmu2")
                nc.vector.scalar_tensor_tensor(out=nmu2[:], in0=mur[:], scalar=-1.0,
                                               in1=mur[:], op0=ALU.mult, op1=ALU.mult)
                ve = TMP.tile([P, 4], F32, tag="ve", name="ve")
                nc.vector.scalar_tensor_tensor(out=ve[:], in0=nmu2[:], scalar=1e-5,
                                               in1=ex2r[:], op0=ALU.add, op1=ALU.add)
                hsh = TMP.tile([P, 4], I32, tag="hsh", name="hsh")
                nc.vector.tensor_scalar(out=hsh[:], in0=ve[:].bitcast(I32), scalar1=1,
                                        scalar2=None, op0=ALU.logical_shift_right)
                nc.vector.tensor_scalar(out=hsh[:], in0=hsh[:], scalar1=-1,
                                        scalar2=MAGIC, op0=ALU.mult, op1=ALU.add)
                y0 = hsh[:].bitcast(F32)
                nt = TMP.tile([P, 4], F32, tag="nt", name="nt")
                nc.vector.tensor_mul(out=nt[:], in0=y0, in1=y0)
                nc.vector.scalar_tensor_tensor(out=nt[:], in0=nt[:], scalar=-0.5,
                                               in1=ve[:], op0=ALU.mult, op1=ALU.mult)
                nc.vector.tensor_scalar_add(out=nt[:], in0=nt[:], scalar1=1.5)
                an_s = TMP.tile([P, 4], BF16, tag="an_s", name="an_s")
                nc.vector.tensor_mul(out=an_s[:], in0=y0, in1=nt[:])
                # token-major rstd broadcast tile [128, ST]
                an_d = DSCR.tile([ST], BF16, tag=f"an{s}", name="an_d")
                nc.sync.dma_start(out=an_d[:].rearrange("(g p) -> p g", p=P),
                                  in_=an_s[:])
                an_free = STR.tile([P, ST], BF16, tag="an_free", name="an_free", bufs=1)
                nc.sync.dma_start(out=an_free[:], in_=bcast_dram(an_d[:], P, ST))
                mu_free = STR.tile([P, ST], BF16, tag="mu_free", name="mu_free")
                nc.sync.dma_start(out=mu_free[:], in_=bcast_dram(nm_d[:], P, ST))

                # LN-normalized activations (centered + scaled), bf16 2x ops
                xsc = STR.tile([P, 2, ST], BF16, tag="xsc", name="xsc", bufs=1)
                mub = bass.AP(tensor=mu_free.tensor, offset=mu_free[:].offset,
                              ap=[list(mu_free[:].ap[0]), [0, 2],
                                  list(mu_free[:].ap[1])])
                nc.vector.tensor_tensor(out=xsc[:], in0=xsb[:], in1=mub, op=ALU.add)
                xss = STR.tile([P, 2, ST], BF16, tag="xss", name="xss")
                anb = bass.AP(tensor=an_free.tensor, offset=an_free[:].offset,
                              ap=[list(an_free[:].ap[0]), [0, 2],
                                  list(an_free[:].ap[1])])
                nc.vector.tensor_tensor(out=xss[:], in0=xsc[:], in1=anb, op=ALU.mult)
                xsss[s] = xss

            def kv_stage(s, part=None):
                xss = xsss[s]
                if part == 1:
                    kv_vpart(s)
                    return
                # t2 (shared kv lora, rank 8) from normalized input
                t2sb = STR.tile([R, ST], BF16, tag="t2sb", name="t2sb")
                kvp = PS.tile([P, 1024], F32, tag="sc", bufs=2, name="kvp")
                t2p = kvp[0:R, 0:ST]
                nc.tensor.matmul(t2p, avT[:, 0, :], xss[:, 0, :], start=True, stop=False)
                nc.tensor.matmul(t2p, avT[:, 1, :], xss[:, 1, :], start=False, stop=True)
                nc.scalar.activation(out=t2sb[:], in_=t2p, func=AF.Copy)

                # k projection: 2 out-chunks x (2 W chunks + rank-8 lora)
                kvq0 = PS.tile([P, 1024], F32, tag="sc", bufs=2, name="kvq0")
                kv_ps = [kvp[:, ST:2 * ST], kvq0[:, 0:ST]]
                for kvoc in range(2):
                    kps = kv_ps[kvoc]
                    nc.tensor.matmul(kps, kvwT[:, 0, kvoc * P:(kvoc + 1) * P],
                                     xss[:, 0, :], start=True, stop=False)
                    nc.tensor.matmul(kps, kvwT[:, 1, kvoc * P:(kvoc + 1) * P],
                                     xss[:, 1, :], start=False, stop=False)
                    nc.tensor.matmul(kps, bvT[:, kvoc, :], t2sb[:],
                                     start=False, stop=True)
                    if kvoc == 0:
                        nc.scalar.activation(out=kts[:, 0, s * ST:(s + 1) * ST],
                                             in_=kps, func=AF.Copy)
                    else:
                        nc.vector.tensor_copy(out=kts[:, 1, s * ST:(s + 1) * ST],
                                              in_=kps)
                kv_t2[s] = t2sb
                if part is None:
                    kv_vpart(s)

            def kv_vpart(s):
                xss = xsss[s]
                t2sb = kv_t2[s]
                vtmp = STR.tile([P, 2, ST], BF16, tag="vtmp", name="vtmp")
                kvq1 = PS.tile([P, 1024], F32, tag="sc", bufs=2, name="kvq1")
                kv_ps = [kvq1[:, 0:ST], kvq1[:, ST:2 * ST]]
                for vc in range(2):
                    kvoc = vc + 2
                    kps = kv_ps[vc]
                    nc.tensor.matmul(kps, kvwT[:, 0, kvoc * P:(kvoc + 1) * P],
                                     xss[:, 0, :], start=True, stop=False)
                    nc.tensor.matmul(kps, kvwT[:, 1, kvoc * P:(kvoc + 1) * P],
                                     xss[:, 1, :], start=False, stop=False)
                    nc.tensor.matmul(kps, bvT[:, kvoc, :], t2sb[:],
                                     start=False, stop=True)
                    if vc == 0:
                        nc.scalar.activation(out=vtmp[:, 0, :], in_=kps,
                                             func=AF.Copy)
                    else:
                        nc.vector.tensor_copy(out=vtmp[:, 1, :], in_=kps)

                # v transpose: 8 PE transposes packed into one psum tile,
                # then one 2x bf16 drain into vsb (ones column pre-set)
                vtp = PS.tile([P, 1024], F32, tag="sc", bufs=2, name="vtp")
                vtb = vtp[:].bitcast(BF16).rearrange("p (u v) -> p u v", v=P)
                for vc in range(2):
                    for u4 in range(4):
                        nc.tensor.transpose(vtb[:, vc * 4 + u4, :],
                                            vtmp[:, vc, u4 * P:(u4 + 1) * P],
                                            ident[:])
                # vsb dims [P, mc 8, vc 2, h' 4, d 33]; iterate (vc, u4, h, d)
                dst = bass.AP(
                    tensor=vsb.tensor, offset=vsb[:, 4 * s, 0, 0, 0].offset,
                    ap=[list(vsb[:].ap[0]),
                        [(DH + 1) * 4, 2], [(DH + 1) * 4 * 2, 4],
                        [DH + 1, 4], [1, DH]])
                nc.vector.tensor_copy(
                    out=dst,
                    in_=vtb[:, 0:8, :].rearrange("p s (h d) -> p s h d", d=DH))

            # =============== q path (needs strip0 only) ===============
            def q_path():
                xs_t = xs_ts[0]
                tqp_t = PS.tile([P, 1024], F32, tag="sc", bufs=2, name="tqp_t")
                for nh in range(2):
                    sl = slice(nh * 512, (nh + 1) * 512)
                    tqp = tqp_t[0:R, sl]
                    nc.tensor.matmul(tqp, aqT[:, 0, :], xs_t[:, 0, sl],
                                     start=True, stop=False)
                    nc.tensor.matmul(tqp, aqT[:, 1, :], xs_t[:, 1, sl],
                                     start=False, stop=True)
                nc.scalar.activation(out=tq[:], in_=tqp_t[0:R, :], func=AF.Copy)
                for oc in range(2):
                    qps = PS.tile([P, 1024], F32, tag="sc", bufs=2, name="qps")
                    for nh in range(2):
                        sl = slice(nh * 512, (nh + 1) * 512)
                        nc.tensor.matmul(qps[:, sl],
                                         qwT[:, 0, oc * P:(oc + 1) * P],
                                         xs_t[:, 0, sl], start=True, stop=False)
                        nc.tensor.matmul(qps[:, sl],
                                         qwT[:, 1, oc * P:(oc + 1) * P],
                                         xs_t[:, 1, sl], start=False, stop=False)
                        nc.tensor.matmul(qps[:, sl], bqT[:, oc, :], tq[:, sl],
                                         start=False, stop=True)
                    nc.scalar.activation(out=qT[:, oc, :], in_=qps[:],
                                         func=AF.Identity, bias=qb[:, oc:oc + 1])

            # =============== attention ===============
            pv_state = {}
            pv_tiles = {}

            def attn_scores_exp(ghalf, g2, mc):
                for h01 in range(2):
                    hh = 2 * g2 + h01          # head within half (0..3)
                    h = 4 * ghalf + hh
                    sc = PS.tile([P, 1024], F32, tag="sc", bufs=2, name="sc")
                    lhsT = kts[32 * hh:32 * hh + 32, ghalf, mc * P:(mc + 1) * P]
                    for nh in range(2):
                        sl = slice(nh * 512, (nh + 1) * 512)
                        nc.tensor.matmul(sc[:, sl], lhsT,
                                         qT[32 * hh:32 * hh + 32, ghalf, sl],
                                         start=True, stop=True,
                                         tile_position=(32 * hh, 0))
                    if _act_tile(h01, mc):
                        nc.scalar.activation(out=pt[:, h, mc, :], in_=sc[:],
                                             func=AF.Exp)
                    else:
                        nc.vector.tensor_scalar(
                            out=pt[:, h, mc, :].bitcast(I16), in0=sc[:],
                            scalar1=SCH_A, scalar2=SCH_B,
                            op0=ALU.mult, op1=ALU.add)

            def attn_pv(ghalf, g2, mc, qcs=range(8)):
                for h01 in range(2):
                    hh = 2 * g2 + h01
                    h = 4 * ghalf + hh
                    for qc in qcs:
                        bank = pv_tiles[(ghalf, qc // 3)]
                        st = pv_state[(ghalf, qc // 3)]
                        nc.tensor.matmul(
                            bank[:, qc % 3, 33 * hh:33 * hh + 33],
                            pt[:, h, mc, qc * P:(qc + 1) * P],
                            vsb[:, mc, ghalf, hh, :],
                            start=(st["n"] == 0),
                            stop=(st["n"] == st["total"] - 1),
                            skip_group_check=True)
                        st["n"] += 1

            def attn_tail(ghalf):
                # reciprocals of the denominator columns, then fused
                # divide + psum drain into attn_sb
                for b3 in range(3):
                    nq = 3 if b3 < 2 else 2
                    bank = pv_tiles[(ghalf, b3)]
                    bap = bank[:].rearrange("p q (h d) -> p q h d", d=DH + 1)
                    r = rec[:, ghalf, b3, 0:nq, :]
                    nc.vector.reciprocal(out=r, in_=bap[:, 0:nq, :, DH])
                    rb = bass.AP(tensor=rec.tensor, offset=r.offset,
                                 ap=[list(d) for d in r.ap] + [[0, DH]])
                    dst = bass.AP(
                        tensor=attn_sb.tensor,
                        offset=attn_sb[:, 3 * b3, ghalf, 0].offset,
                        ap=[list(attn_sb[:].ap[0]), [2 * P, nq], [DH, 4], [1, DH]])
                    nc.vector.tensor_tensor(out=dst, in0=bap[:, 0:nq, :, 0:DH],
                                            in1=rb, op=ALU.mult)

            def attn_transpose(ghalf, qp):
                # attn_sb [q, c-block] -> outT [c, q]; 2 PE transposes packed
                # per sc tile (bf16), one 2x drain (split ACT/DVE)
                tp = PS.tile([P, 1024], F32, tag="sc", bufs=2, name="tp")
                tb = tp[:].bitcast(BF16).rearrange("p (u v) -> p u v", v=P)
                for i in range(2):
                    nc.tensor.transpose(tb[:, i, :],
                                        attn_sb[:, 2 * qp + i, ghalf, :],
                                        ident[:])
                dst = bass.AP(
                    tensor=outT.tensor,
                    offset=outT[:, ghalf, 2 * qp * P].offset,
                    ap=[list(outT[:].ap[0]), [P, 2], [1, P]])
                if qp % 2 == 0:
                    nc.scalar.activation(out=dst, in_=tb[:, 0:2, :], func=AF.Copy)
                else:
                    nc.vector.tensor_copy(out=dst, in_=tb[:, 0:2, :])

            def proj_out(t8):
                pp_t = PS.tile([P, 3, 132], F32, tag="pv", bufs=4, name="pp_t")
                pp = pp_t[:].bitcast(F32).rearrange("p a b -> p (a b)")[:, 0:C]
                nc.tensor.matmul(pp, outT[:, 0, t8 * P:(t8 + 1) * P],
                                 pwT[:, 0, :], start=True, stop=False)
                nc.tensor.matmul(pp, outT[:, 1, t8 * P:(t8 + 1) * P],
                                 pwT[:, 1, :], start=False, stop=False)
                nc.tensor.matmul(pp, onesq[:], pbrow, start=False, stop=True)
                fin = FIN.tile([P, C], F32, tag="fin", name="fin", bufs=3)
                if t8 % 2 == 0:
                    nc.scalar.activation(out=fin[:], in_=pp, func=AF.Copy)
                else:
                    nc.vector.tensor_copy(out=fin[:], in_=pp)
                nc.sync.dma_start(out=out_d[t8 * P:(t8 + 1) * P, :], in_=fin[:])

            # =============== emission schedule ===============
            def pv_alloc(ghalf):
                for b3 in range(3):
                    t = PS.tile([P, 3, 132], F32, tag="pv", bufs=4,
                                name=f"pv{ghalf}{b3}")
                    pv_tiles[(ghalf, b3)] = t
                    nq = 3 if b3 < 2 else 2
                    pv_state[(ghalf, b3)] = {"n": 0, "total": nq * 4 * 8}

            conv_stage(0)
            stats_a(0)
            conv_stage(1)
            stats_a(1)
            q_path()                 # PE-heavy; runs while LN chains complete
            stats_b(0)
            kv_stage(0)
            pv_alloc(0)
            pv_alloc(1)

            # software-pipelined attention: pv matmuls lag scores/exp by one
            # step so the in-order PE queue never parks on an exp result.
            # Both ghalves' strip-0 steps run before the strip-1 kv
            # projection (whose LN DMA chain is slow); gh1's pv work is
            # restricted to its conflict-free psum bank until gh0's banks
            # drain at tail(0).
            steps = ([(0, g2, mc) for g2 in range(2) for mc in range(4)]
                     + [(1, g2, mc) for g2 in range(2) for mc in range(4)]
                     + ["kv1a",
                        (0, 0, 4)]
                     + ["kv1b"]
                     + [(0, g2, mc) for g2 in range(2) for mc in range(4, 8)][1:]
                     + ["tail0"]
                     + [(1, g2, mc) for g2 in range(2) for mc in range(4, 8)])
            pvq = []            # lagged pv batches
            gh1_stash = []      # gh1 batches awaiting banks 1,2
            transq = []         # deferred gh0 transposes
            gh0_open = True
            cnt = 0

            def pump(budget=1):
                n = 0
                while pvq and n < budget:
                    it = pvq[0]
                    if it[0] == 0:
                        attn_pv(*it)
                        pvq.pop(0)
                    elif gh0_open:
                        attn_pv(*it, qcs=range(3))
                        gh1_stash.append(it)
                        pvq.pop(0)
                    else:
                        attn_pv(*it)
                        pvq.pop(0)
                    n += 1

            for item in steps:
                if item == "kv1a":
                    kv_stage(1, part=0)
                    continue
                if item == "kv1b":
                    kv_stage(1, part=1)
                    continue
                if item == "tail0":
                    pump(budget=8)      # flush all remaining gh0 batches
                    attn_tail(0)
                    gh0_open = False
                    transq = [(0, qp) for qp in range(4)]
                    continue
                attn_scores_exp(*item)
                pump()
                if not gh0_open and gh1_stash:
                    it = gh1_stash.pop(0)
                    attn_pv(*it, qcs=range(3, 8))
                cnt += 1
                if cnt == 2:
                    stats_b(1)
                if transq and cnt % 2 == 0:
                    attn_transpose(*transq.pop(0))
                pvq.append(item)
            pump(budget=8)
            while gh1_stash:
                attn_pv(*gh1_stash.pop(0), qcs=range(3, 8))
            while transq:
                attn_transpose(*transq.pop(0))
            attn_tail(1)
            for qp in range(4):
                attn_transpose(1, qp)
                proj_out(2 * qp)
                proj_out(2 * qp + 1)

    nc.finalize()
    return nc


def _prep_shared(q_w, q_b, kv_w, kv_b, proj_w, proj_b, a_q, b_q, a_v, b_v,
                 sr_w, sr_b, ln_g, ln_b):
    f32 = np.float32

    def chunkT(w):  # [in, out] -> [128, n_in_chunks * out] (chunk-major cols)
        wt = np.ascontiguousarray(np.asarray(w, f32).T)
        ic, oc = wt.shape
        return np.ascontiguousarray(
            wt.reshape(ic // 128, 128, oc).transpose(1, 0, 2)).reshape(128, -1)

    kv_w = np.asarray(kv_w, f32)
    a_v = np.asarray(a_v, f32)
    b_v = np.asarray(b_v, f32)
    g = np.asarray(ln_g, f32)
    bb = np.asarray(ln_b, f32)
    proj_w = np.asarray(proj_w, f32)
    # fold LayerNorm gamma into kv/a_v weights; LN mean/rstd applied on-chip;
    # k-side beta constants dropped (softmax shift invariance), v-side folded
    # into the projection bias. 1/sqrt(dh) folded into q weights.
    Wg = kv_w * g[None, :]
    Avg = a_v * g[None, :]
    wbt = kv_w @ bb + np.asarray(kv_b, f32)
    dconst = b_v @ (a_v @ bb)
    wv_const = wbt[C:] + dconst
    pb_eff = np.asarray(proj_b, f32) + proj_w @ wv_const

    srwT = np.asarray(sr_w, f32).transpose(1, 2, 3, 0).reshape(2, 128, 4, C)
    srwT = np.ascontiguousarray(srwT.transpose(1, 0, 2, 3)).reshape(128, -1)
    bqT = (np.asarray(b_q, f32) * SCALE).T.reshape(R, 2 * 128)
    bvT2 = b_v.T.reshape(R, 2, 128)      # [R, kv-half-chunk, 128]
    bvT = np.zeros((R, 4 * 128), f32)
    for kvoc in range(4):
        bvT[:, kvoc * 128:(kvoc + 1) * 128] = bvT2[:, kvoc % 2, :]

    wpk = np.zeros((128, _WCOLS), f32)
    wpk[:, _QW0:_QW1] = chunkT(np.asarray(q_w, f32) * SCALE)
    wpk[:, _KV0:_KV1] = chunkT(Wg)
    wpk[:, _PW0:_PW1] = chunkT(proj_w)
    wpk[:, _SR0:_SR1] = srwT
    wpk[:, _AQ0:_AQ1] = chunkT(a_q)
    wpk[:, _AV0:_AV1] = chunkT(Avg)
    wpk[0:R, _BQ0:_BQ1] = bqT
    wpk[0:R, _BV0:_BV1] = bvT
    wpk[0:1, _PB0:_PB1] = pb_eff.reshape(1, C)

    fpk = np.zeros((128, 4), f32)
    fpk[:, 0:2] = (np.asarray(q_b, f32) * SCALE).reshape(2, 128).T
    fpk[:, 2:4] = np.asarray(sr_b, f32).reshape(2, 128).T
    return dict(wpk=np.ascontiguousarray(wpk).astype(BF16NP),
                fpk=np.ascontiguousarray(fpk))


def kernel(x, q_w, q_b, kv_w, kv_b, proj_w, proj_b, a_q, b_q, a_v, b_v,
           sr_w, sr_b, ln_g, ln_b, H, W):
    from concourse.bass_utils import run_bass_kernel_spmd

    x = np.asarray(x, np.float32)
    assert x.shape == (B, N, C) and int(H) == 64 and int(W) == 64

    if "nc" not in _CACHE:
        _CACHE["nc"] = _build_program()
    nc = _CACHE["nc"]

    shared = _prep_shared(q_w, q_b, kv_w, kv_b, proj_w, proj_b, a_q, b_q,
                          a_v, b_v, sr_w, sr_b, ln_g, ln_b)
    in_maps = []
    for c in range(NCORES):
        b, j = c // 4, c % 4
        xb = np.roll(x[b], -NB * j, axis=0)          # own block at rows 0:1024
        xT = np.ascontiguousarray(xb.T.astype(BF16NP))  # [256, 4096]
        xT = np.ascontiguousarray(
            xT.reshape(2, 128, N).transpose(1, 0, 2))   # [128, 2, 4096]
        in_maps.append(dict(shared, xT=xT))

    res = run_bass_kernel_spmd(nc, in_maps, list(range(NCORES)))
    out = np.empty((B, N, C), np.float32)
    for c in range(NCORES):
        b, j = c // 4, c % 4
        out[b, NB * j:NB * (j + 1)] = res.results[c]["out"]
    return out
